# revision 58
# baseline (speedup 1.0000x reference)
"""Multi-head attention (B=2, S=2048, D=1024, H=16, dk=dv=64) on 8 TRN2 NeuronCores.

Sharding: core c -> (batch b = c//4, head-group g = c%4, 4 heads each).
Each core computes q/k/v projections for its 4 heads (weight-column shard),
attention over its batch, and a partial output projection over its 256
channels (weight-row shard of Wo).  The host sums the 4 partial outputs per
batch at unshard time (the "all-reduce after the output projection").

Perf notes (v10, ~151us median vs 165us v3 baseline; run-to-run spread
is +-4us from the free-running HAM window and DMA phase):
  * Inputs are host-swizzled so each SBUF partition's slab is one
    contiguous DRAM run (4-8KB DMA descriptors; the natural [D, S]
    layout fragments to 1KB rows and the stream goes descriptor-rate
    bound).  The stream is bandwidth-bound from ~8us on; wk and xk
    block0 are dj-chunked so the first kproj matmul issues ~10us.
  * Attention can't start before the 7.5MB prerequisite lands (~28us),
    so the prefix runs BOTH m-chunks of kproj and qproj half-0 in DMA
    arrival order to keep the PE dense through the window.  The first
    unit runs all 8 score/exp pairs before its AVs (pend=8) and pops
    one vproj pair before every other AV — vproj pair (j,j+1) must
    fully pop before AV j is emitted or the in-order PE deadlocks.
  * kT is zero-padded per head to 128 contraction rows (kTp): 64-row
    quadrant score matmuls ran at 317ns (ldweights can't background-
    load into a (64,128) tile); (128,128) tiles sustain 216ns with LDW
    hidden.  The zero rows select the head from the shared 128-row qT.
  * Softmax denominator: 65th "ones" column of V -> PSUM row 64.
    reciprocal() is free-size bound (~6.4ns/elem) so it must run on
    the [128,8] transposed form (1024 unique elems, 167ns).  Mid-phase
    units: denom row -> [128,8] via one SBUF->SBUF reshape DMA, recip,
    DRAM bounce, 64-partition broadcast — all on the Act HWDGE queue
    (never head-of-line blocks the sync queue's bulk stream).  The
    FINAL unit uses a DMA-free PE chain instead (transpose x8 ->
    recip -> transpose-back -> selection-matmul broadcast into the
    freed psc bank): the 3 DMA trigger latencies (~4us) would idle the
    PE past the HAM window and the tail would run cold.
  * Tail: the three pre-run chunks' m=0 matmuls are emitted before any
    m=1 (an early m=1 stalls the in-order PE queue on the final
    normalize and blocks the pre-runs queued behind it); qc12/13
    recycle the psw banks freed by qc8/9's evacs.
  * Key-padding mask applied by host-side compaction (1002 -> 1024 of
    2048 keys); 1/sqrt(dk) folded into Wq/bq; exp without
    max-subtraction (|s| ~ 10 fits bf16).
"""
import numpy as np
import ml_dtypes

B, S, D = 2, 2048, 1024
H, DK, DV = 16, 64, 64
SCALE = float(np.sqrt(DK))
NCORES = 8
GROUPS = 4           # head-groups (cores per batch)
HPG = H // GROUPS    # heads per core = 4
CH = HPG * DK        # channels per core = 256
MC = CH // 128       # c-chunks = 2
DJ = D // 128        # contraction chunks = 8
NQC = S // 128       # 16
P = 128

_BUILD_CACHE = {}
LAST_RESULTS = None  # test harness can read exec_time_ns etc. from here


def _bf16(a: np.ndarray) -> np.ndarray:
    return np.ascontiguousarray(a, dtype=np.float32).astype(ml_dtypes.bfloat16)


def _build(n_kp: int):
    """Build + schedule the per-core Bass program for a padded key count."""
    import concourse.bass as bass  # noqa: F401
    from concourse import bacc, tile, mybir

    DT = mybir.dt
    F32, BF16 = DT.float32, DT.bfloat16
    AF = mybir.ActivationFunctionType
    ALU = mybir.AluOpType

    NJ = n_kp // P                      # 128-wide k chunks
    NKB = n_kp // 512                   # 512-wide k blocks (n_kp % 512 == 0)

    nc = bacc.Bacc("TRN2", target_bir_lowering=False, debug=False,
                   num_devices=NCORES)

    # activation/weight tensors are host-swizzled so each SBUF partition's
    # slab is one contiguous DRAM run (4-8KB DMA descriptors; the natural
    # [D, S] layout fragments to 1KB rows and the input stream becomes
    # descriptor-rate-bound, not bandwidth-bound)
    xqT = nc.dram_tensor("xqT", [S // 512, P, DJ, 512], BF16, kind="ExternalInput")
    xkT = nc.dram_tensor("xkT", [NKB, P, DJ, 512], BF16, kind="ExternalInput")
    xvT = nc.dram_tensor("xvT", [NKB, P, DJ, 512], BF16, kind="ExternalInput")
    wqT = nc.dram_tensor("wqT", [P, DJ, CH], BF16, kind="ExternalInput")
    wkT = nc.dram_tensor("wkT", [P, DJ, CH], BF16, kind="ExternalInput")
    wvT = nc.dram_tensor("wvT", [P, DJ, CH], BF16, kind="ExternalInput")
    woT = nc.dram_tensor("woT", [P, MC, D], BF16, kind="ExternalInput")
    bq = nc.dram_tensor("bq", [CH], F32, kind="ExternalInput")
    bk = nc.dram_tensor("bk", [CH], F32, kind="ExternalInput")
    bv = nc.dram_tensor("bv", [CH], F32, kind="ExternalInput")
    valid = nc.dram_tensor("valid", [n_kp], F32, kind="ExternalInput")
    ident = nc.dram_tensor("ident", [P, P], F32, kind="ExternalInput")
    sel8 = nc.dram_tensor("sel8", [8, 8, DV], F32, kind="ExternalInput")
    # bf16 partials (summed in fp32 on the host): halves the output DMA
    # stream and the end-of-kernel drain; costs ~0.1e-2 of rel err.
    out = nc.dram_tensor("out", [S, D], BF16, kind="ExternalOutput")

    with tile.TileContext(nc) as tc:
        with (
            tc.tile_pool(name="persist", bufs=1) as pp,
            tc.tile_pool(name="exps", bufs=10) as ep,
            tc.tile_pool(name="scratch", bufs=3) as scr,
            tc.tile_pool(name="outs", bufs=3) as op,
            tc.tile_pool(name="norm", bufs=2) as npool,
            tc.tile_pool(name="smalls", bufs=4) as smalls,
            tc.tile_pool(name="cu", bufs=2) as cu,
            tc.tile_pool(name="dscr", bufs=2, space="DRAM") as dscr,
            tc.tile_pool(name="psw", bufs=2, space="PSUM") as psw,   # ST (4 banks)
            tc.tile_pool(name="psc", bufs=1, space="PSUM") as psc,   # ctx (2 banks)
            tc.tile_pool(name="pso", bufs=2, space="PSUM") as pso,   # proj/outproj (2 banks)
        ):
            # ---- persistent SBUF ------------------------------------------
            wq_sb = pp.tile([P, DJ, CH], BF16, name="wq_sb")
            wk_sb = pp.tile([P, DJ, CH], BF16, name="wk_sb")
            wv_sb = pp.tile([P, DJ, CH], BF16, name="wv_sb")
            wo_sb = pp.tile([P, MC, D], BF16, name="wo_sb")
            bq_sb = pp.tile([P, MC], F32, name="bq_sb")
            bk_sb = pp.tile([P, MC], F32, name="bk_sb")
            qT_sb = pp.tile([P, MC, S], BF16, name="qT_sb")
            # per-head kT, zero-padded to 128 contraction rows: head 2m
            # lives in rows 0:64 (rows 64:128 zero), head 2m+1 in rows
            # 64:128 (rows 0:64 zero) -- the zeros select the head out of
            # the full-128-row qT chunk in the score matmul.
            kTp = pp.tile([P, HPG, n_kp], BF16, name="kTp")
            vaug = pp.tile([P, NJ, HPG, DV + 1], BF16, name="vaug")
            ctxN = pp.tile([P, MC, S], BF16, name="ctxN")
            xk_sb = pp.tile([P, NKB, DJ, 512], BF16, name="xk_sb")
            xv_sb = pp.tile([P, NKB, DJ, 512], BF16, name="xv_sb")
            xq_sb = pp.tile([P, S // 512, DJ, 512], BF16, name="xq_sb")
            bv_rep = pp.tile([P, CH], F32, name="bv_rep")
            valid_sb = pp.tile([P, NJ], F32, name="valid_sb")
            valid_bf = pp.tile([P, NJ], BF16, name="valid_bf")
            ident_sb = pp.tile([P, P], F32, name="ident_sb")
            sel_sb = pp.tile([P, 8, DV], F32, name="sel_sb")
            ones1 = pp.tile([P, 1], F32, name="ones1")
            nc.vector.memset(ones1[:], 1.0)

            # zero the pad halves of kTp once, before any kproj evac
            for h in range(HPG):
                z0, z1 = (64, 128) if h % 2 == 0 else (0, 64)
                nc.vector.memset(kTp[z0:z1, h, :], 0.0)

            # ---- DMA stream: issue order == consumption order -------------
            # (each dma_start's descriptors spread across all 16 HWDGE
            # queues, so arrival order tracks issue order at ~330 GB/s)
            # tiny per-partition tensors (128 descriptors at the per-desc
            # floor) ride the Act HWDGE queue so they never stall the sync
            # queue's bulk stream
            nc.scalar.dma_start(out=bk_sb[:], in_=bk.ap().rearrange("(m p) -> p m", p=P))
            nc.scalar.dma_start(out=bq_sb[:], in_=bq.ap().rearrange("(m p) -> p m", p=P))
            nc.scalar.dma_start(out=valid_sb[:], in_=valid.ap().rearrange("(j p) -> p j", p=P))
            nc.scalar.dma_start(out=ident_sb[:], in_=ident.ap())
            nc.scalar.dma_start(out=sel_sb[0:8], in_=sel8.ap())
            # wk and xk block0 are dj-chunked so the first kproj matmul can
            # issue ~1.5us after the stream starts moving
            nc.sync.dma_start(out=wk_sb[:, 0:2], in_=wkT.ap()[:, 0:2])
            nc.sync.dma_start(out=xk_sb[:, 0, 0:2], in_=xkT.ap()[0][:, 0:2])
            nc.sync.dma_start(out=wk_sb[:, 2:DJ], in_=wkT.ap()[:, 2:DJ])
            for dj0 in range(2, DJ, 2):
                nc.sync.dma_start(out=xk_sb[:, 0, dj0:dj0 + 2],
                                  in_=xkT.ap()[0][:, dj0:dj0 + 2])
            nc.sync.dma_start(out=wq_sb[:], in_=wqT.ap())
            nc.sync.dma_start(out=xq_sb[:, 0], in_=xqT.ap()[0])
            for kb in range(1, NKB):
                nc.sync.dma_start(out=xk_sb[:, kb], in_=xkT.ap()[kb])
            nc.sync.dma_start(out=xq_sb[:, 1], in_=xqT.ap()[1])
            nc.sync.dma_start(out=wv_sb[:], in_=wvT.ap())
            nc.gpsimd.dma_start(out=bv_rep[:], in_=bv.ap()[None, :].partition_broadcast(P))
            for kb in range(NKB):
                nc.sync.dma_start(out=xv_sb[:, kb], in_=xvT.ap()[kb])
            for qb in range(2, S // 512):
                nc.sync.dma_start(out=xq_sb[:, qb], in_=xqT.ap()[qb])
            nc.sync.dma_start(out=wo_sb[:], in_=woT.ap())

            nc.vector.tensor_copy(out=valid_bf[:], in_=valid_sb[:])

            # ---- projection emitters (steps = one PE matmul or one evac) --
            def kproj_steps(m, kb):
                c0 = kb * 512
                ps = pso.tile([P, 512], DT.float32, tag="po")
                steps = []
                for dj in range(DJ):
                    def mm(dj=dj, ps=ps, kb=kb):
                        nc.tensor.matmul(
                            ps[:, :],
                            lhsT=wk_sb[:, dj, m * P:(m + 1) * P],
                            rhs=xk_sb[:, kb, dj, :],
                            start=(dj == 0), stop=(dj == DJ - 1))
                    steps.append(mm)

                def evac(ps=ps, c0=c0):
                    # split per head so each lands in its padded rows
                    nc.vector.tensor_scalar(
                        out=kTp[0:64, 2 * m, c0:c0 + 512], in0=ps[0:64, :],
                        scalar1=bk_sb[0:64, m:m + 1], scalar2=None, op0=ALU.add)
                    nc.vector.tensor_scalar(
                        out=kTp[64:128, 2 * m + 1, c0:c0 + 512], in0=ps[64:128, :],
                        scalar1=bk_sb[64:128, m:m + 1], scalar2=None, op0=ALU.add)
                steps.append(evac)
                return steps

            def vproj_steps(jp):
                """one pair of 128-wide k chunks [jp, jp+1]"""
                jn = min(2, NJ - jp)
                ps = pso.tile([P, 512], DT.float32, tag="po")
                steps = []
                for ji in range(jn):
                    j = jp + ji
                    kb, sub = divmod(j, 4)
                    for dj in range(DJ):
                        def mm(kb=kb, sub=sub, ji=ji, dj=dj, ps=ps):
                            nc.tensor.matmul(
                                ps[:, ji * CH:(ji + 1) * CH],
                                lhsT=xv_sb[:, kb, dj, sub * P:(sub + 1) * P],
                                rhs=wv_sb[:, dj, :],
                                start=(dj == 0), stop=(dj == DJ - 1))
                        steps.append(mm)

                def post(ps=ps, jp=jp, jn=jn):
                    for ji in range(jn):
                        j = jp + ji
                        vst = scr.tile([P, CH], DT.float32, tag="s")
                        nc.vector.tensor_tensor(out=vst[:], in0=ps[:, ji * CH:(ji + 1) * CH],
                                                in1=bv_rep[:], op=ALU.add)
                        nc.vector.tensor_scalar(
                            out=vaug[:, j, :, 0:DV],
                            in0=vst[:].rearrange("p (h d) -> p h d", h=HPG),
                            scalar1=valid_sb[:, j:j + 1], scalar2=None, op0=ALU.mult)
                        for h in range(HPG):
                            nc.gpsimd.tensor_copy(out=vaug[:, j, h, DV:DV + 1],
                                                  in_=valid_bf[:, j:j + 1])
                steps.append(post)
                return steps

            def qproj_steps(qb, m):
                c0, c1 = qb * 512, (qb + 1) * 512
                ps = pso.tile([P, 512], DT.float32, tag="po")
                steps = []
                for dj in range(DJ):
                    def mm(dj=dj, ps=ps):
                        nc.tensor.matmul(
                            ps[:, :],
                            lhsT=wq_sb[:, dj, m * P:(m + 1) * P],
                            rhs=xq_sb[:, qb, dj, :],
                            start=(dj == 0), stop=(dj == DJ - 1))
                    steps.append(mm)

                def evac(ps=ps):
                    nc.vector.tensor_scalar(
                        out=qT_sb[:, m, c0:c1], in0=ps[:, :],
                        scalar1=bq_sb[:, m:m + 1], scalar2=None, op0=ALU.add)
                steps.append(evac)
                return steps

            # ---- output projection (interleaved into half-1 attention) ----
            def outproj_steps(qc, evac_engine="vector"):
                steps = []
                stage = op.tile([P, 1024], BF16, tag="o", name=f"og{qc}")
                for n2 in range(2):
                    ps = pso.tile([P, 512], DT.float32, tag="po",
                                  name=f"ops{qc}_{n2}")
                    for m in range(MC):
                        def mm(ps=ps, n2=n2, m=m, qc=qc, stage=stage,
                               last=(m == MC - 1), fin=(n2 == 1 and m == MC - 1),
                               eng=evac_engine):
                            nc.tensor.matmul(
                                ps[:, :],
                                lhsT=ctxN[:, m, qc * P:(qc + 1) * P],
                                rhs=wo_sb[:, m, n2 * 512:(n2 + 1) * 512],
                                start=(m == 0), stop=(m == MC - 1))
                            if last:
                                sl = slice(n2 * 512, (n2 + 1) * 512)
                                # "both": n0 on vector, n1 on scalar (parallel
                                # evac for the tail where ACT is free)
                                if eng == "scalar" or (eng == "both" and n2 == 1):
                                    nc.scalar.copy(out=stage[:, sl], in_=ps[:])
                                else:
                                    nc.vector.tensor_copy(out=stage[:, sl], in_=ps[:])
                            if fin:
                                nc.sync.dma_start(
                                    out=out.ap()[qc * P:(qc + 1) * P, :],
                                    in_=stage[:])
                        steps.append(mm)
                return steps

            # ---- attention unit: scores^T -> exp -> ctx^T (+denominator) --
            ilq = []

            def emit_attention(half, h, islots=1, pend=1, flush_islots=None,
                               fast_norm=False):
                q0 = half * 1024
                m, po = h // 2, (h % 2) * 64
                ctx_ps = psc.tile([P, 1024], DT.float32, tag="ctx",
                                  name=f"ctx{half}{h}")

                def emit_av(j, ex):
                    for qq in range(2):
                        nc.tensor.matmul(
                            ctx_ps[0:DV + 1, qq * 512:(qq + 1) * 512],
                            lhsT=vaug[:, j, h, :],
                            rhs=ex[:, qq * 512:(qq + 1) * 512],
                            start=(j == 0), stop=(j == NJ - 1))

                pend_q = []
                for j in range(NJ):
                    st = psw.tile([P, 1024], DT.float32, tag="ps",
                                  name=f"st{half}{h}{j}")
                    for qq in range(2):
                        nc.tensor.matmul(
                            st[:, qq * 512:(qq + 1) * 512],
                            lhsT=kTp[:, h, j * P:(j + 1) * P],
                            rhs=qT_sb[:, m, q0 + qq * 512:q0 + (qq + 1) * 512],
                            start=True, stop=True)
                    ex = ep.tile([P, 1024], BF16, tag="e", name=f"ex{half}{h}{j}")
                    nc.scalar.activation(out=ex[:], in_=st[:], func=AF.Exp)
                    for _ in range(islots):
                        if ilq:
                            ilq.pop(0)()
                    pend_q.append((j, ex))
                    if len(pend_q) > pend:
                        emit_av(*pend_q.pop(0))
                for fi, (j, ex) in enumerate(pend_q):
                    if flush_islots is not None:
                        for _ in range(flush_islots[fi]):
                            if ilq:
                                ilq.pop(0)()
                    emit_av(j, ex)

                # evac ctx early (frees the psc PSUM banks for the next
                # unit).  Normalize: reciprocal() is element-count bound
                # (~10 elem/ns) and runs fine at partition base 64, so take
                # it straight off the PSUM denominator row (1024 unique
                # elements, ~0.3us, concurrent with the evac), then one
                # row-contiguous DRAM bounce (1 descriptor) feeds the
                # 64-partition broadcast (64 x 4KB descriptors).  The
                # normalize DMAs ride the Act HWDGE queue so they never
                # head-of-line block the sync queue's in/out stream.
                ctxU = cu.tile([P, 1024], DT.float32, tag="cu",
                               name=f"cu{half}{h}")
                nc.vector.tensor_copy(out=ctxU[0:DV + 1, :], in_=ctx_ps[0:DV + 1, :])
                if fast_norm:
                    # DMA-free finale (the DMA chain's 3 trigger latencies
                    # cost ~4us and let HAM re-throttle): PE transposes the
                    # denom row to [128,8], reciprocal there, PE transposes
                    # back and broadcasts via selection matmuls into a free
                    # psw bank; PE stays warm through the whole chain.
                    assert po == 0
                    denT = pso.tile([P, 512], DT.float32, tag="po", name="denT")
                    for qb8 in range(8):
                        nc.tensor.transpose(
                            denT[:, qb8:qb8 + 1],
                            in_=ctxU[DV:DV + 1, qb8 * P:(qb8 + 1) * P],
                            identity=ones1[DV:DV + 1, :])
                    rcq = smalls.tile([P, 8], F32, tag="rcq")
                    nc.vector.reciprocal(out=rcq[:], in_=denT[:, 0:8])
                    r8ps = pso.tile([P, 512], DT.float32, tag="po", name="r8ps")
                    nc.tensor.transpose(r8ps[0:8, 0:P], in_=rcq[:],
                                        identity=ident_sb[:, 0:P])
                    r8 = smalls.tile([P, P], F32, tag="r8")
                    nc.vector.tensor_copy(out=r8[0:8, :], in_=r8ps[0:8, 0:P])
                    # psc is free once the ctxU evac lands (guaranteed by
                    # this chain's dependencies), and the psw banks stay
                    # free for the tail chunks' m=0 pre-runs
                    recps = psc.tile([P, 1024], DT.float32, tag="ctx",
                                     name="recps")
                    for qb8 in range(8):
                        nc.tensor.matmul(
                            recps[0:DV, qb8 * P:(qb8 + 1) * P],
                            lhsT=sel_sb[0:8, qb8, :], rhs=r8[0:8, :],
                            start=True, stop=True)
                    nc.vector.tensor_tensor(out=ctxN[0:64, m, q0:q0 + 1024],
                                            in0=ctxU[0:64, :],
                                            in1=recps[0:64, :], op=ALU.mult)
                    return
                rsq = smalls.tile([P, 8], F32, tag="rsq")
                nc.scalar.dma_start(out=rsq[:], in_=ctxU[DV:DV + 1, :])
                rcq = smalls.tile([P, 8], F32, tag="rcq")
                nc.vector.reciprocal(out=rcq[:], in_=rsq[:])
                rb2 = dscr.tile([1, 1024], F32, tag="rb2")
                nc.scalar.dma_start(out=rb2.rearrange("o (p a) -> (o p) a", p=P),
                                    in_=rcq[:])
                rec = npool.tile([P, 1024], F32, tag="rc", name=f"rc{half}{h}")
                nc.scalar.dma_start(out=rec[0:64, :],
                                    in_=rb2[0][None, :].partition_broadcast(64))
                if po == 0:
                    nc.vector.tensor_tensor(out=ctxN[0:64, m, q0:q0 + 1024],
                                            in0=ctxU[0:64, :],
                                            in1=rec[0:64, :], op=ALU.mult)
                else:
                    tmp = scr.tile([P, 1024], BF16, tag="s", name=f"tm{half}{h}")
                    nc.vector.tensor_tensor(out=tmp[0:64, :],
                                            in0=ctxU[0:64, :],
                                            in1=rec[0:64, :], op=ALU.mult)
                    nc.sync.dma_start(out=ctxN[64:128, m, q0:q0 + 1024],
                                      in_=tmp[0:64, :])

            # warm-up dummies: matmuls on already-arrived weight bytes into
            # scratch PSUM nobody reads.  They bridge the prefix's DMA
            # arrival gaps so the HAM clock-gate (re-throttles to 1.2GHz
            # after ~3.4us idle) keeps the PE at 2.4GHz — without them the
            # first ~3.4us of matmuls after every gap run at half clock.
            def warmup(n):
                wup = psw.tile([P, 1024], DT.float32, tag="ps", name="wup")
                for _ in range(n):
                    nc.tensor.matmul(wup[:, 0:CH], lhsT=wk_sb[:, 0, 0:P],
                                     rhs=wk_sb[:, 0, :], start=True, stop=True)

            # ---- PE prefix: tracks the DMA arrival order, reusing every
            # arrived byte for both m-chunks before moving on (the stream
            # is bandwidth-bound from ~8us; attention can't start before
            # ~7.5MB have landed, so the prefix must fill that window) ----
            warmup(14)
            for m, kb in ((1, 0), (0, 0)):
                for s_ in kproj_steps(m, kb):
                    s_()
            warmup(6)
            for s_ in qproj_steps(0, 1):
                s_()
            for s_ in qproj_steps(0, 0):
                s_()
            for m, kb in ((1, 1), (0, 1)):
                for s_ in kproj_steps(m, kb):
                    s_()
            warmup(8)
            for s_ in qproj_steps(1, 1):
                s_()

            # ---- deferred projections ride the attention interleave queue -
            # ordering constraint: vproj pair (j, j+1) must fully pop before
            # AV j is emitted (PE executes in order; a stalled AV would
            # deadlock against vproj matmuls queued behind it).  The first
            # unit therefore runs all 8 score/exp pairs first (pend=8) and
            # pops one vproj pair before every other AV during the flush —
            # which also aligns the AVs with xv's DMA arrival.
            for jp in range(0, NJ, 2):
                ilq.extend(vproj_steps(jp))
            ilq.extend(qproj_steps(1, 0))
            for qb in range(2, 4):          # half-1 q: m0 first (h1 is the
                ilq.extend(qproj_steps(qb, 0))   # first half-1 unit)
            for qb in range(2, 4):
                ilq.extend(qproj_steps(qb, 1))

            # half 0: m=1 heads (3, 2) first so outproj's m=1 chunk is ready
            # early; ends on h0 (po=0: no shift DMA in its normalize chain).
            emit_attention(0, 3, islots=0, pend=NJ,
                           flush_islots=[17, 0] * (NJ // 2))
            for h, isl in zip((2, 1, 0), (3, 2, 1)):
                emit_attention(0, h, islots=isl)
            assert not ilq, f"{len(ilq)} interleave items left after half 0"
            # half 1: outproj for half-0 q rows interleaved; first unit takes
            # none (the last half-0 normalize chain lands around its end).
            for qc in range(8):
                ilq.extend(outproj_steps(qc))
            for h, isl in zip((1, 3, 0, 2), (0, 2, 2, 1)):
                emit_attention(1, h, islots=isl, fast_norm=(h == 2))
            for _ in range(len(ilq)):
                ilq.pop(0)()
            # tail: half-1 q rows; evac halves in parallel on vector+scalar.
            # The first two chunks accumulate in psw tiles (free after the
            # last score matmul) and a third in pso, and ALL their ready
            # m=0 matmuls are emitted before any m=1 — an m=1 emitted early
            # would stall the in-order PE queue on the final normalize and
            # block the other pre-runs behind it.
            tail_ps = {}
            for i, qc in enumerate(range(8, 11)):
                if i < 2:
                    psf = psw.tile([P, 1024], DT.float32, tag="ps",
                                   name=f"tps{qc}")
                    tail_ps[qc] = [psf[:, 0:512], psf[:, 512:1024]]
                else:
                    tail_ps[qc] = [pso.tile([P, 512], DT.float32, tag="po",
                                            name=f"ops{qc}_{n2}")[:, :]
                                   for n2 in range(2)]
                for n2 in range(2):
                    nc.tensor.matmul(
                        tail_ps[qc][n2],
                        lhsT=ctxN[:, 0, qc * P:(qc + 1) * P],
                        rhs=wo_sb[:, 0, n2 * 512:(n2 + 1) * 512],
                        start=True, stop=False)
            for qc in range(8, NQC):
                stage = op.tile([P, 1024], BF16, tag="o", name=f"og{qc}")
                qsl = slice(qc * P, (qc + 1) * P)
                if qc not in tail_ps and qc in (12, 13):
                    # recycle the psw banks freed by qc8/9's evacs — a
                    # 4-deep tail pipeline instead of ping-ponging pso
                    psf = psw.tile([P, 1024], DT.float32, tag="ps",
                                   name=f"tps{qc}")
                    tail_ps[qc] = None
                    halves = [psf[:, 0:512], psf[:, 512:1024]]
                for n2 in range(2):
                    wsl = slice(n2 * 512, (n2 + 1) * 512)
                    if tail_ps.get(qc) is not None:
                        ps = tail_ps[qc][n2]
                        nc.tensor.matmul(ps, lhsT=ctxN[:, 1, qsl],
                                         rhs=wo_sb[:, 1, wsl],
                                         start=False, stop=True)
                    else:
                        if qc in (12, 13):
                            ps = halves[n2]
                        else:
                            ps = pso.tile([P, 512], DT.float32, tag="po",
                                          name=f"ops{qc}_{n2}")[:, :]
                        for m in range(MC):
                            nc.tensor.matmul(ps, lhsT=ctxN[:, m, qsl],
                                             rhs=wo_sb[:, m, wsl],
                                             start=(m == 0), stop=(m == MC - 1))
                    # each 512-half flies as soon as its evac lands — the
                    # final teardown barrier waits on the last DMA, so
                    # don't hold the n0 half for the n1 evac
                    if n2 == 0:
                        nc.vector.tensor_copy(out=stage[:, wsl], in_=ps)
                    else:
                        nc.scalar.copy(out=stage[:, wsl], in_=ps)
                    nc.sync.dma_start(out=out.ap()[qc * P:(qc + 1) * P, wsl],
                                      in_=stage[:, wsl])

    nc.compile()
    return nc


def _ensure_axon_hooks():
    """bass_utils imports antenv.axon_hooks when tracing; this image's antenv
    lacks it. Provide it, backed by the ctypes NTFF hook when available."""
    import sys
    import types
    try:
        import antenv.axon_hooks  # noqa: F401
        return
    except ImportError:
        pass
    hook = None
    try:
        from trn_agent_boot.trn_boot import _ntff_profile_via_ctypes
        hook = _ntff_profile_via_ctypes("/opt/axon/libaxon_pjrt.so")
    except Exception:
        hook = None
    mod = types.ModuleType("antenv.axon_hooks")
    mod._hook = hook
    mod.get_axon_ntff_profile_hook = lambda: mod._hook
    mod.set_axon_ntff_profile_hook = lambda h: setattr(mod, "_hook", h)
    sys.modules["antenv.axon_hooks"] = mod


def kernel(Q, K, V, atte_mask_out, Wq, bq, Wk, bk, Wv, bv, Wo, bo):
    import jax  # noqa: F401  (must be imported first so the axon backend registers)
    from concourse.bass_utils import run_bass_kernel_spmd
    global LAST_RESULTS
    _ensure_axon_hooks()

    Q = np.asarray(Q); K = np.asarray(K); V = np.asarray(V)
    mask = np.asarray(atte_mask_out).reshape(B, S)
    Wq = np.asarray(Wq); Wk = np.asarray(Wk); Wv = np.asarray(Wv); Wo = np.asarray(Wo)
    bq = np.asarray(bq); bk = np.asarray(bk); bv = np.asarray(bv); bo = np.asarray(bo)

    keep = [np.flatnonzero(~mask[b]) for b in range(B)]
    n_kp = max(512, max(((len(ix) + 511) // 512) * 512 for ix in keep))

    def swz_x(xT):       # [D, n] f32 -> [n//512, P, DJ, 512] bf16
        n = xT.shape[1]
        return _bf16(xT.reshape(DJ, P, n // 512, 512).transpose(2, 1, 0, 3))

    def swz_w(wT):       # [D, CH] f32 -> [P, DJ, CH] bf16
        return _bf16(wT.reshape(DJ, P, CH).transpose(1, 0, 2))

    # per-batch packed bf16 tensors
    xqT, xkT, xvT, validv = [], [], [], []
    for b in range(B):
        ix = keep[b]
        xqT.append(swz_x(Q[b].T))
        kk = np.zeros((D, n_kp), np.float32)
        vv = np.zeros((D, n_kp), np.float32)
        kk[:, :len(ix)] = K[b][ix].T
        vv[:, :len(ix)] = V[b][ix].T
        xkT.append(swz_x(kk))
        xvT.append(swz_x(vv))
        va = np.zeros(n_kp, np.float32)
        va[:len(ix)] = 1.0
        validv.append(va)

    ident_np = np.eye(P, dtype=np.float32)
    sel8_np = np.ascontiguousarray(
        np.repeat(np.eye(8, dtype=np.float32)[:, :, None], DV, axis=2),
        dtype=np.float32)

    in_maps = []
    for c in range(NCORES):
        b, g = c // GROUPS, c % GROUPS
        sl = slice(g * CH, (g + 1) * CH)
        in_maps.append({
            "ident": ident_np, "sel8": sel8_np,
            "xqT": xqT[b], "xkT": xkT[b], "xvT": xvT[b],
            "wqT": swz_w(Wq[sl].T / SCALE),
            "wkT": swz_w(Wk[sl].T),
            "wvT": swz_w(Wv[sl].T),
            "woT": _bf16(Wo[:, sl].T.reshape(MC, P, D).transpose(1, 0, 2)),
            "bq": np.ascontiguousarray(bq[sl] / SCALE, np.float32),
            "bk": np.ascontiguousarray(bk[sl], np.float32),
            "bv": np.ascontiguousarray(bv[sl], np.float32),
            "valid": validv[b],
        })

    if n_kp not in _BUILD_CACHE:
        _BUILD_CACHE[n_kp] = _build(n_kp)
    nc = _BUILD_CACHE[n_kp]

    res = run_bass_kernel_spmd(nc, in_maps, core_ids=list(range(NCORES)))
    LAST_RESULTS = res

    full = np.zeros((B, S, D), np.float32)
    full += bo.astype(np.float32)
    for c in range(NCORES):
        full[c // GROUPS] += np.asarray(res.results[c]["out"], dtype=np.float32)
    return full


# revision 59
# speedup vs baseline: 1.0475x; 1.0475x over previous
"""Multi-head attention (B=2, S=2048, D=1024, H=16, dk=dv=64) on 8 TRN2 NeuronCores.

Sharding: core c -> (batch b = c//4, head-group g = c%4, 4 heads each).
Each core computes q/k/v projections for its 4 heads (weight-column shard),
attention over its batch, and a partial output projection over its 256
channels (weight-row shard of Wo).  The host sums the 4 partial outputs per
batch at unshard time (the "all-reduce after the output projection").

Perf notes (v10, ~151us median vs 165us v3 baseline; run-to-run spread
is +-4us from the free-running HAM window and DMA phase):
  * Inputs are host-swizzled so each SBUF partition's slab is one
    contiguous DRAM run (4-8KB DMA descriptors; the natural [D, S]
    layout fragments to 1KB rows and the stream goes descriptor-rate
    bound).  The stream is bandwidth-bound from ~8us on; wk and xk
    block0 are dj-chunked so the first kproj matmul issues ~10us.
  * Attention can't start before the 7.5MB prerequisite lands (~28us),
    so the prefix runs BOTH m-chunks of kproj and qproj half-0 in DMA
    arrival order to keep the PE dense through the window.  The first
    unit runs all 8 score/exp pairs before its AVs (pend=8) and pops
    one vproj pair before every other AV — vproj pair (j,j+1) must
    fully pop before AV j is emitted or the in-order PE deadlocks.
  * kT is zero-padded per head to 128 contraction rows (kTp): 64-row
    quadrant score matmuls ran at 317ns (ldweights can't background-
    load into a (64,128) tile); (128,128) tiles sustain 216ns with LDW
    hidden.  The zero rows select the head from the shared 128-row qT.
  * Softmax denominator: 65th "ones" column of V -> PSUM row 64.
    reciprocal() is free-size bound (~6.4ns/elem) so it must run on
    the [128,8] transposed form (1024 unique elems, 167ns).  Mid-phase
    units: denom row -> [128,8] via one SBUF->SBUF reshape DMA, recip,
    DRAM bounce, 64-partition broadcast — all on the Act HWDGE queue
    (never head-of-line blocks the sync queue's bulk stream).  The
    FINAL unit uses a DMA-free PE chain instead (transpose x8 ->
    recip -> transpose-back -> selection-matmul broadcast into the
    freed psc bank): the 3 DMA trigger latencies (~4us) would idle the
    PE past the HAM window and the tail would run cold.
  * Tail: the three pre-run chunks' m=0 matmuls are emitted before any
    m=1 (an early m=1 stalls the in-order PE queue on the final
    normalize and blocks the pre-runs queued behind it); qc12/13
    recycle the psw banks freed by qc8/9's evacs.
  * Key-padding mask applied by host-side compaction (1002 -> 1024 of
    2048 keys); 1/sqrt(dk) folded into Wq/bq; exp without
    max-subtraction (|s| ~ 10 fits bf16).
"""
import numpy as np
import ml_dtypes

B, S, D = 2, 2048, 1024
H, DK, DV = 16, 64, 64
SCALE = float(np.sqrt(DK))
NCORES = 8
GROUPS = 4           # head-groups (cores per batch)
HPG = H // GROUPS    # heads per core = 4
CH = HPG * DK        # channels per core = 256
MC = CH // 128       # c-chunks = 2
DJ = D // 128        # contraction chunks = 8
NQC = S // 128       # 16
P = 128

_BUILD_CACHE = {}
LAST_RESULTS = None  # test harness can read exec_time_ns etc. from here


def _bf16(a: np.ndarray) -> np.ndarray:
    return np.ascontiguousarray(a, dtype=np.float32).astype(ml_dtypes.bfloat16)


def _build(n_kp: int):
    """Build + schedule the per-core Bass program for a padded key count."""
    import concourse.bass as bass  # noqa: F401
    from concourse import bacc, tile, mybir

    DT = mybir.dt
    F32, BF16 = DT.float32, DT.bfloat16
    AF = mybir.ActivationFunctionType
    ALU = mybir.AluOpType

    NJ = n_kp // P                      # 128-wide k chunks
    NKB = n_kp // 512                   # 512-wide k blocks (n_kp % 512 == 0)

    nc = bacc.Bacc("TRN2", target_bir_lowering=False, debug=False,
                   num_devices=NCORES)

    # activation/weight tensors are host-swizzled so each SBUF partition's
    # slab is one contiguous DRAM run (4-8KB DMA descriptors; the natural
    # [D, S] layout fragments to 1KB rows and the input stream becomes
    # descriptor-rate-bound, not bandwidth-bound)
    xqT = nc.dram_tensor("xqT", [S // 512, P, DJ, 512], BF16, kind="ExternalInput")
    xkT = nc.dram_tensor("xkT", [NKB, P, DJ, 512], BF16, kind="ExternalInput")
    xvT = nc.dram_tensor("xvT", [NKB, P, DJ, 512], BF16, kind="ExternalInput")
    wqT = nc.dram_tensor("wqT", [P, DJ, CH], BF16, kind="ExternalInput")
    wkT = nc.dram_tensor("wkT", [P, DJ, CH], BF16, kind="ExternalInput")
    wvT = nc.dram_tensor("wvT", [P, DJ, CH], BF16, kind="ExternalInput")
    woT = nc.dram_tensor("woT", [P, MC, D], BF16, kind="ExternalInput")
    bq = nc.dram_tensor("bq", [CH], F32, kind="ExternalInput")
    bk = nc.dram_tensor("bk", [CH], F32, kind="ExternalInput")
    bv = nc.dram_tensor("bv", [CH], F32, kind="ExternalInput")
    valid = nc.dram_tensor("valid", [n_kp], F32, kind="ExternalInput")
    ident = nc.dram_tensor("ident", [P, P], F32, kind="ExternalInput")
    sel8 = nc.dram_tensor("sel8", [8, 8, DV], F32, kind="ExternalInput")
    # bf16 partials (summed in fp32 on the host): halves the output DMA
    # stream and the end-of-kernel drain; costs ~0.1e-2 of rel err.
    out = nc.dram_tensor("out", [S, D], BF16, kind="ExternalOutput")

    with tile.TileContext(nc) as tc:
        with (
            tc.tile_pool(name="persist", bufs=1) as pp,
            tc.tile_pool(name="exps", bufs=10) as ep,
            tc.tile_pool(name="scratch", bufs=3) as scr,
            tc.tile_pool(name="outs", bufs=3) as op,
            tc.tile_pool(name="norm", bufs=2) as npool,
            tc.tile_pool(name="smalls", bufs=4) as smalls,
            tc.tile_pool(name="cu", bufs=2) as cu,
            tc.tile_pool(name="dscr", bufs=2, space="DRAM") as dscr,
            tc.tile_pool(name="psw", bufs=2, space="PSUM") as psw,   # ST (4 banks)
            tc.tile_pool(name="psc", bufs=1, space="PSUM") as psc,   # ctx (2 banks)
            tc.tile_pool(name="pso", bufs=2, space="PSUM") as pso,   # proj/outproj (2 banks)
        ):
            # ---- persistent SBUF ------------------------------------------
            wq_sb = pp.tile([P, DJ, CH], BF16, name="wq_sb")
            wk_sb = pp.tile([P, DJ, CH], BF16, name="wk_sb")
            wv_sb = pp.tile([P, DJ, CH], BF16, name="wv_sb")
            wo_sb = pp.tile([P, MC, D], BF16, name="wo_sb")
            bq_sb = pp.tile([P, MC], F32, name="bq_sb")
            bk_sb = pp.tile([P, MC], F32, name="bk_sb")
            qT_sb = pp.tile([P, MC, S], BF16, name="qT_sb")
            # per-head kT, zero-padded to 128 contraction rows: head 2m
            # lives in rows 0:64 (rows 64:128 zero), head 2m+1 in rows
            # 64:128 (rows 0:64 zero) -- the zeros select the head out of
            # the full-128-row qT chunk in the score matmul.
            kTp = pp.tile([P, HPG, n_kp], BF16, name="kTp")
            vaug = pp.tile([P, NJ, HPG, DV + 1], BF16, name="vaug")
            ctxN = pp.tile([P, MC, S], BF16, name="ctxN")
            xk_sb = pp.tile([P, NKB, DJ, 512], BF16, name="xk_sb")
            xv_sb = pp.tile([P, NKB, DJ, 512], BF16, name="xv_sb")
            xq_sb = pp.tile([P, S // 512, DJ, 512], BF16, name="xq_sb")
            bv_rep = pp.tile([P, CH], F32, name="bv_rep")
            valid_sb = pp.tile([P, NJ], F32, name="valid_sb")
            valid_bf = pp.tile([P, NJ], BF16, name="valid_bf")
            ident_sb = pp.tile([P, P], F32, name="ident_sb")
            sel_sb = pp.tile([P, 8, DV], F32, name="sel_sb")
            ones1 = pp.tile([P, 1], F32, name="ones1")
            nc.vector.memset(ones1[:], 1.0)

            # zero the pad halves of kTp once, before any kproj evac
            for h in range(HPG):
                z0, z1 = (64, 128) if h % 2 == 0 else (0, 64)
                nc.vector.memset(kTp[z0:z1, h, :], 0.0)

            # ---- DMA stream: issue order == consumption order -------------
            # (each dma_start's descriptors spread across all 16 HWDGE
            # queues, so arrival order tracks issue order at ~330 GB/s)
            # tiny per-partition tensors (128 descriptors at the per-desc
            # floor) ride the Act HWDGE queue so they never stall the sync
            # queue's bulk stream
            nc.scalar.dma_start(out=bk_sb[:], in_=bk.ap().rearrange("(m p) -> p m", p=P))
            nc.scalar.dma_start(out=bq_sb[:], in_=bq.ap().rearrange("(m p) -> p m", p=P))
            nc.scalar.dma_start(out=valid_sb[:], in_=valid.ap().rearrange("(j p) -> p j", p=P))
            nc.scalar.dma_start(out=ident_sb[:], in_=ident.ap())
            nc.scalar.dma_start(out=sel_sb[0:8], in_=sel8.ap())
            # wk and xk block0 are dj-chunked so the first kproj matmul can
            # issue ~1.5us after the stream starts moving
            nc.sync.dma_start(out=wk_sb[:, 0:2], in_=wkT.ap()[:, 0:2])
            nc.sync.dma_start(out=xk_sb[:, 0, 0:2], in_=xkT.ap()[0][:, 0:2])
            nc.sync.dma_start(out=wk_sb[:, 2:DJ], in_=wkT.ap()[:, 2:DJ])
            for dj0 in range(2, DJ, 2):
                nc.sync.dma_start(out=xk_sb[:, 0, dj0:dj0 + 2],
                                  in_=xkT.ap()[0][:, dj0:dj0 + 2])
            nc.sync.dma_start(out=wq_sb[:], in_=wqT.ap())
            nc.sync.dma_start(out=xq_sb[:, 0], in_=xqT.ap()[0])
            for kb in range(1, NKB):
                nc.sync.dma_start(out=xk_sb[:, kb], in_=xkT.ap()[kb])
            nc.sync.dma_start(out=xq_sb[:, 1], in_=xqT.ap()[1])
            nc.sync.dma_start(out=wv_sb[:], in_=wvT.ap())
            nc.gpsimd.dma_start(out=bv_rep[:], in_=bv.ap()[None, :].partition_broadcast(P))
            for kb in range(NKB):
                nc.sync.dma_start(out=xv_sb[:, kb], in_=xvT.ap()[kb])
            for qb in range(2, S // 512):
                nc.sync.dma_start(out=xq_sb[:, qb], in_=xqT.ap()[qb])
            nc.sync.dma_start(out=wo_sb[:], in_=woT.ap())

            nc.vector.tensor_copy(out=valid_bf[:], in_=valid_sb[:])

            # ---- projection emitters (steps = one PE matmul or one evac) --
            def kproj_steps(m, kb):
                c0 = kb * 512
                ps = pso.tile([P, 512], DT.float32, tag="po")
                steps = []
                for dj in range(DJ):
                    def mm(dj=dj, ps=ps, kb=kb):
                        nc.tensor.matmul(
                            ps[:, :],
                            lhsT=wk_sb[:, dj, m * P:(m + 1) * P],
                            rhs=xk_sb[:, kb, dj, :],
                            start=(dj == 0), stop=(dj == DJ - 1))
                    steps.append(mm)

                def evac(ps=ps, c0=c0):
                    # split per head so each lands in its padded rows
                    nc.vector.tensor_scalar(
                        out=kTp[0:64, 2 * m, c0:c0 + 512], in0=ps[0:64, :],
                        scalar1=bk_sb[0:64, m:m + 1], scalar2=None, op0=ALU.add)
                    nc.vector.tensor_scalar(
                        out=kTp[64:128, 2 * m + 1, c0:c0 + 512], in0=ps[64:128, :],
                        scalar1=bk_sb[64:128, m:m + 1], scalar2=None, op0=ALU.add)
                steps.append(evac)
                return steps

            def vproj_steps(jp):
                """one pair of 128-wide k chunks [jp, jp+1]"""
                jn = min(2, NJ - jp)
                ps = pso.tile([P, 512], DT.float32, tag="po")
                steps = []
                for ji in range(jn):
                    j = jp + ji
                    kb, sub = divmod(j, 4)
                    for dj in range(DJ):
                        def mm(kb=kb, sub=sub, ji=ji, dj=dj, ps=ps):
                            nc.tensor.matmul(
                                ps[:, ji * CH:(ji + 1) * CH],
                                lhsT=xv_sb[:, kb, dj, sub * P:(sub + 1) * P],
                                rhs=wv_sb[:, dj, :],
                                start=(dj == 0), stop=(dj == DJ - 1))
                        steps.append(mm)

                def post(ps=ps, jp=jp, jn=jn):
                    for ji in range(jn):
                        j = jp + ji
                        vst = scr.tile([P, CH], DT.float32, tag="s")
                        nc.vector.tensor_tensor(out=vst[:], in0=ps[:, ji * CH:(ji + 1) * CH],
                                                in1=bv_rep[:], op=ALU.add)
                        nc.vector.tensor_scalar(
                            out=vaug[:, j, :, 0:DV],
                            in0=vst[:].rearrange("p (h d) -> p h d", h=HPG),
                            scalar1=valid_sb[:, j:j + 1], scalar2=None, op0=ALU.mult)
                        for h in range(HPG):
                            nc.gpsimd.tensor_copy(out=vaug[:, j, h, DV:DV + 1],
                                                  in_=valid_bf[:, j:j + 1])
                steps.append(post)
                return steps

            def qproj_steps(qb, m):
                c0, c1 = qb * 512, (qb + 1) * 512
                ps = pso.tile([P, 512], DT.float32, tag="po")
                steps = []
                for dj in range(DJ):
                    def mm(dj=dj, ps=ps):
                        nc.tensor.matmul(
                            ps[:, :],
                            lhsT=wq_sb[:, dj, m * P:(m + 1) * P],
                            rhs=xq_sb[:, qb, dj, :],
                            start=(dj == 0), stop=(dj == DJ - 1))
                    steps.append(mm)

                def evac(ps=ps):
                    nc.vector.tensor_scalar(
                        out=qT_sb[:, m, c0:c1], in0=ps[:, :],
                        scalar1=bq_sb[:, m:m + 1], scalar2=None, op0=ALU.add)
                steps.append(evac)
                return steps

            # ---- output projection (interleaved into half-1 attention) ----
            def outproj_steps(qc, evac_engine="vector"):
                steps = []
                stage = op.tile([P, 1024], BF16, tag="o", name=f"og{qc}")
                for n2 in range(2):
                    ps = pso.tile([P, 512], DT.float32, tag="po",
                                  name=f"ops{qc}_{n2}")
                    for m in range(MC):
                        def mm(ps=ps, n2=n2, m=m, qc=qc, stage=stage,
                               last=(m == MC - 1), fin=(n2 == 1 and m == MC - 1),
                               eng=evac_engine):
                            nc.tensor.matmul(
                                ps[:, :],
                                lhsT=ctxN[:, m, qc * P:(qc + 1) * P],
                                rhs=wo_sb[:, m, n2 * 512:(n2 + 1) * 512],
                                start=(m == 0), stop=(m == MC - 1))
                            if last:
                                sl = slice(n2 * 512, (n2 + 1) * 512)
                                # "both": n0 on vector, n1 on scalar (parallel
                                # evac for the tail where ACT is free)
                                if eng == "scalar" or (eng == "both" and n2 == 1):
                                    nc.scalar.copy(out=stage[:, sl], in_=ps[:])
                                else:
                                    nc.vector.tensor_copy(out=stage[:, sl], in_=ps[:])
                            if fin:
                                nc.sync.dma_start(
                                    out=out.ap()[qc * P:(qc + 1) * P, :],
                                    in_=stage[:])
                        steps.append(mm)
                return steps

            # ---- attention unit: scores^T -> exp -> ctx^T (+denominator) --
            ilq = []

            def emit_attention(half, h, islots=1, pend=1, flush_islots=None,
                               fast_norm=False):
                q0 = half * 1024
                m, po = h // 2, (h % 2) * 64
                ctx_ps = psc.tile([P, 1024], DT.float32, tag="ctx",
                                  name=f"ctx{half}{h}")

                def emit_av(j, ex):
                    for qq in range(2):
                        nc.tensor.matmul(
                            ctx_ps[0:DV + 1, qq * 512:(qq + 1) * 512],
                            lhsT=vaug[:, j, h, :],
                            rhs=ex[:, qq * 512:(qq + 1) * 512],
                            start=(j == 0), stop=(j == NJ - 1))

                pend_q = []
                for j in range(NJ):
                    st = psw.tile([P, 1024], DT.float32, tag="ps",
                                  name=f"st{half}{h}{j}")
                    for qq in range(2):
                        nc.tensor.matmul(
                            st[:, qq * 512:(qq + 1) * 512],
                            lhsT=kTp[:, h, j * P:(j + 1) * P],
                            rhs=qT_sb[:, m, q0 + qq * 512:q0 + (qq + 1) * 512],
                            start=True, stop=True)
                    ex = ep.tile([P, 1024], BF16, tag="e", name=f"ex{half}{h}{j}")
                    nc.scalar.activation(out=ex[:], in_=st[:], func=AF.Exp)
                    for _ in range(islots):
                        if ilq:
                            ilq.pop(0)()
                    pend_q.append((j, ex))
                    if len(pend_q) > pend:
                        emit_av(*pend_q.pop(0))
                for fi, (j, ex) in enumerate(pend_q):
                    if flush_islots is not None:
                        for _ in range(flush_islots[fi]):
                            if ilq:
                                ilq.pop(0)()
                    emit_av(j, ex)

                # evac ctx early (frees the psc PSUM banks for the next
                # unit).  Normalize: reciprocal() is element-count bound
                # (~10 elem/ns) and runs fine at partition base 64, so take
                # it straight off the PSUM denominator row (1024 unique
                # elements, ~0.3us, concurrent with the evac), then one
                # row-contiguous DRAM bounce (1 descriptor) feeds the
                # 64-partition broadcast (64 x 4KB descriptors).  The
                # normalize DMAs ride the Act HWDGE queue so they never
                # head-of-line block the sync queue's in/out stream.
                ctxU = cu.tile([P, 1024], DT.float32, tag="cu",
                               name=f"cu{half}{h}")
                nc.vector.tensor_copy(out=ctxU[0:DV + 1, :], in_=ctx_ps[0:DV + 1, :])
                if fast_norm:
                    # DMA-free finale (the DMA chain's 3 trigger latencies
                    # cost ~4us and let HAM re-throttle): PE transposes the
                    # denom row to [128,8], reciprocal there, PE transposes
                    # back and broadcasts via selection matmuls into a free
                    # psw bank; PE stays warm through the whole chain.
                    assert po == 0
                    denT = pso.tile([P, 512], DT.float32, tag="po", name="denT")
                    for qb8 in range(8):
                        nc.tensor.transpose(
                            denT[:, qb8:qb8 + 1],
                            in_=ctxU[DV:DV + 1, qb8 * P:(qb8 + 1) * P],
                            identity=ones1[DV:DV + 1, :])
                    rcq = smalls.tile([P, 8], F32, tag="rcq")
                    nc.vector.reciprocal(out=rcq[:], in_=denT[:, 0:8])
                    r8ps = pso.tile([P, 512], DT.float32, tag="po", name="r8ps")
                    nc.tensor.transpose(r8ps[0:8, 0:P], in_=rcq[:],
                                        identity=ident_sb[:, 0:P])
                    r8 = smalls.tile([P, P], F32, tag="r8")
                    nc.vector.tensor_copy(out=r8[0:8, :], in_=r8ps[0:8, 0:P])
                    # psc is free once the ctxU evac lands (guaranteed by
                    # this chain's dependencies), and the psw banks stay
                    # free for the tail chunks' m=0 pre-runs
                    recps = psc.tile([P, 1024], DT.float32, tag="ctx",
                                     name="recps")
                    for qb8 in range(8):
                        nc.tensor.matmul(
                            recps[0:DV, qb8 * P:(qb8 + 1) * P],
                            lhsT=sel_sb[0:8, qb8, :], rhs=r8[0:8, :],
                            start=True, stop=True)
                    nc.vector.tensor_tensor(out=ctxN[0:64, m, q0:q0 + 1024],
                                            in0=ctxU[0:64, :],
                                            in1=recps[0:64, :], op=ALU.mult)
                    return
                # these three triggers carry multi-us semaphore WAITS (the
                # broadcast waits the bounce's completion); on the Act
                # queue they head-of-line block the EXP instructions in
                # the strict-FIFO engine queue — exp cadence IS the phase
                # cadence.  On sync they only delay out-chunk DMAs, which
                # have ~30us of slack.
                rsq = smalls.tile([P, 8], F32, tag="rsq")
                nc.sync.dma_start(out=rsq[:], in_=ctxU[DV:DV + 1, :])
                rcq = smalls.tile([P, 8], F32, tag="rcq")
                nc.vector.reciprocal(out=rcq[:], in_=rsq[:])
                rb2 = dscr.tile([1, 1024], F32, tag="rb2")
                nc.sync.dma_start(out=rb2.rearrange("o (p a) -> (o p) a", p=P),
                                  in_=rcq[:])
                rec = npool.tile([P, 1024], F32, tag="rc", name=f"rc{half}{h}")
                nc.sync.dma_start(out=rec[0:64, :],
                                  in_=rb2[0][None, :].partition_broadcast(64))
                if po == 0:
                    nc.vector.tensor_tensor(out=ctxN[0:64, m, q0:q0 + 1024],
                                            in0=ctxU[0:64, :],
                                            in1=rec[0:64, :], op=ALU.mult)
                else:
                    tmp = scr.tile([P, 1024], BF16, tag="s", name=f"tm{half}{h}")
                    nc.vector.tensor_tensor(out=tmp[0:64, :],
                                            in0=ctxU[0:64, :],
                                            in1=rec[0:64, :], op=ALU.mult)
                    nc.sync.dma_start(out=ctxN[64:128, m, q0:q0 + 1024],
                                      in_=tmp[0:64, :])

            # warm-up dummies: matmuls on already-arrived weight bytes into
            # scratch PSUM nobody reads.  They bridge the prefix's DMA
            # arrival gaps so the HAM clock-gate (re-throttles to 1.2GHz
            # after ~3.4us idle) keeps the PE at 2.4GHz — without them the
            # first ~3.4us of matmuls after every gap run at half clock.
            def warmup(n):
                wup = psw.tile([P, 1024], DT.float32, tag="ps", name="wup")
                for _ in range(n):
                    nc.tensor.matmul(wup[:, 0:CH], lhsT=wk_sb[:, 0, 0:P],
                                     rhs=wk_sb[:, 0, :], start=True, stop=True)

            # ---- PE prefix: tracks the DMA arrival order, reusing every
            # arrived byte for both m-chunks before moving on (the stream
            # is bandwidth-bound from ~8us; attention can't start before
            # ~7.5MB have landed, so the prefix must fill that window) ----
            warmup(14)
            for m, kb in ((1, 0), (0, 0)):
                for s_ in kproj_steps(m, kb):
                    s_()
            warmup(6)
            for s_ in qproj_steps(0, 1):
                s_()
            for s_ in qproj_steps(0, 0):
                s_()
            for m, kb in ((1, 1), (0, 1)):
                for s_ in kproj_steps(m, kb):
                    s_()
            warmup(8)
            for s_ in qproj_steps(1, 1):
                s_()

            # ---- deferred projections ride the attention interleave queue -
            # ordering constraint: vproj pair (j, j+1) must fully pop before
            # AV j is emitted (PE executes in order; a stalled AV would
            # deadlock against vproj matmuls queued behind it).  The first
            # unit therefore runs all 8 score/exp pairs first (pend=8) and
            # pops one vproj pair before every other AV during the flush —
            # which also aligns the AVs with xv's DMA arrival.
            for jp in range(0, NJ, 2):
                ilq.extend(vproj_steps(jp))
            ilq.extend(qproj_steps(1, 0))
            for qb in range(2, 4):          # half-1 q: m0 first (h1 is the
                ilq.extend(qproj_steps(qb, 0))   # first half-1 unit)
            for qb in range(2, 4):
                ilq.extend(qproj_steps(qb, 1))

            # half 0: m=1 heads (3, 2) first so outproj's m=1 chunk is ready
            # early; ends on h0 (po=0: no shift DMA in its normalize chain).
            emit_attention(0, 3, islots=0, pend=NJ,
                           flush_islots=[17, 0] * (NJ // 2))
            for h, isl in zip((2, 1, 0), (3, 2, 1)):
                emit_attention(0, h, islots=isl)
            assert not ilq, f"{len(ilq)} interleave items left after half 0"
            # half 1: outproj for half-0 q rows interleaved; first unit takes
            # none (the last half-0 normalize chain lands around its end).
            for qc in range(8):
                ilq.extend(outproj_steps(qc))
            for h, isl in zip((1, 3, 0, 2), (0, 2, 2, 1)):
                emit_attention(1, h, islots=isl, fast_norm=(h == 2))
            for _ in range(len(ilq)):
                ilq.pop(0)()
            # tail: half-1 q rows; evac halves in parallel on vector+scalar.
            # The first two chunks accumulate in psw tiles (free after the
            # last score matmul) and a third in pso, and ALL their ready
            # m=0 matmuls are emitted before any m=1 — an m=1 emitted early
            # would stall the in-order PE queue on the final normalize and
            # block the other pre-runs behind it.
            tail_ps = {}
            for i, qc in enumerate(range(8, 11)):
                if i < 2:
                    psf = psw.tile([P, 1024], DT.float32, tag="ps",
                                   name=f"tps{qc}")
                    tail_ps[qc] = [psf[:, 0:512], psf[:, 512:1024]]
                else:
                    tail_ps[qc] = [pso.tile([P, 512], DT.float32, tag="po",
                                            name=f"ops{qc}_{n2}")[:, :]
                                   for n2 in range(2)]
                for n2 in range(2):
                    nc.tensor.matmul(
                        tail_ps[qc][n2],
                        lhsT=ctxN[:, 0, qc * P:(qc + 1) * P],
                        rhs=wo_sb[:, 0, n2 * 512:(n2 + 1) * 512],
                        start=True, stop=False)
            for qc in range(8, NQC):
                stage = op.tile([P, 1024], BF16, tag="o", name=f"og{qc}")
                qsl = slice(qc * P, (qc + 1) * P)
                if qc not in tail_ps and qc in (12, 13):
                    # recycle the psw banks freed by qc8/9's evacs — a
                    # 4-deep tail pipeline instead of ping-ponging pso
                    psf = psw.tile([P, 1024], DT.float32, tag="ps",
                                   name=f"tps{qc}")
                    tail_ps[qc] = None
                    halves = [psf[:, 0:512], psf[:, 512:1024]]
                for n2 in range(2):
                    wsl = slice(n2 * 512, (n2 + 1) * 512)
                    if tail_ps.get(qc) is not None:
                        ps = tail_ps[qc][n2]
                        nc.tensor.matmul(ps, lhsT=ctxN[:, 1, qsl],
                                         rhs=wo_sb[:, 1, wsl],
                                         start=False, stop=True)
                    else:
                        if qc in (12, 13):
                            ps = halves[n2]
                        else:
                            ps = pso.tile([P, 512], DT.float32, tag="po",
                                          name=f"ops{qc}_{n2}")[:, :]
                        for m in range(MC):
                            nc.tensor.matmul(ps, lhsT=ctxN[:, m, qsl],
                                             rhs=wo_sb[:, m, wsl],
                                             start=(m == 0), stop=(m == MC - 1))
                    # each 512-half flies as soon as its evac lands — the
                    # final teardown barrier waits on the last DMA, so
                    # don't hold the n0 half for the n1 evac
                    if n2 == 0:
                        nc.vector.tensor_copy(out=stage[:, wsl], in_=ps)
                    else:
                        nc.scalar.copy(out=stage[:, wsl], in_=ps)
                    nc.sync.dma_start(out=out.ap()[qc * P:(qc + 1) * P, wsl],
                                      in_=stage[:, wsl])

    nc.compile()
    return nc


def _ensure_axon_hooks():
    """bass_utils imports antenv.axon_hooks when tracing; this image's antenv
    lacks it. Provide it, backed by the ctypes NTFF hook when available."""
    import sys
    import types
    try:
        import antenv.axon_hooks  # noqa: F401
        return
    except ImportError:
        pass
    hook = None
    try:
        from trn_agent_boot.trn_boot import _ntff_profile_via_ctypes
        hook = _ntff_profile_via_ctypes("/opt/axon/libaxon_pjrt.so")
    except Exception:
        hook = None
    mod = types.ModuleType("antenv.axon_hooks")
    mod._hook = hook
    mod.get_axon_ntff_profile_hook = lambda: mod._hook
    mod.set_axon_ntff_profile_hook = lambda h: setattr(mod, "_hook", h)
    sys.modules["antenv.axon_hooks"] = mod


def kernel(Q, K, V, atte_mask_out, Wq, bq, Wk, bk, Wv, bv, Wo, bo):
    import jax  # noqa: F401  (must be imported first so the axon backend registers)
    from concourse.bass_utils import run_bass_kernel_spmd
    global LAST_RESULTS
    _ensure_axon_hooks()

    Q = np.asarray(Q); K = np.asarray(K); V = np.asarray(V)
    mask = np.asarray(atte_mask_out).reshape(B, S)
    Wq = np.asarray(Wq); Wk = np.asarray(Wk); Wv = np.asarray(Wv); Wo = np.asarray(Wo)
    bq = np.asarray(bq); bk = np.asarray(bk); bv = np.asarray(bv); bo = np.asarray(bo)

    keep = [np.flatnonzero(~mask[b]) for b in range(B)]
    n_kp = max(512, max(((len(ix) + 511) // 512) * 512 for ix in keep))

    def swz_x(xT):       # [D, n] f32 -> [n//512, P, DJ, 512] bf16
        n = xT.shape[1]
        return _bf16(xT.reshape(DJ, P, n // 512, 512).transpose(2, 1, 0, 3))

    def swz_w(wT):       # [D, CH] f32 -> [P, DJ, CH] bf16
        return _bf16(wT.reshape(DJ, P, CH).transpose(1, 0, 2))

    # per-batch packed bf16 tensors
    xqT, xkT, xvT, validv = [], [], [], []
    for b in range(B):
        ix = keep[b]
        xqT.append(swz_x(Q[b].T))
        kk = np.zeros((D, n_kp), np.float32)
        vv = np.zeros((D, n_kp), np.float32)
        kk[:, :len(ix)] = K[b][ix].T
        vv[:, :len(ix)] = V[b][ix].T
        xkT.append(swz_x(kk))
        xvT.append(swz_x(vv))
        va = np.zeros(n_kp, np.float32)
        va[:len(ix)] = 1.0
        validv.append(va)

    ident_np = np.eye(P, dtype=np.float32)
    sel8_np = np.ascontiguousarray(
        np.repeat(np.eye(8, dtype=np.float32)[:, :, None], DV, axis=2),
        dtype=np.float32)

    in_maps = []
    for c in range(NCORES):
        b, g = c // GROUPS, c % GROUPS
        sl = slice(g * CH, (g + 1) * CH)
        in_maps.append({
            "ident": ident_np, "sel8": sel8_np,
            "xqT": xqT[b], "xkT": xkT[b], "xvT": xvT[b],
            "wqT": swz_w(Wq[sl].T / SCALE),
            "wkT": swz_w(Wk[sl].T),
            "wvT": swz_w(Wv[sl].T),
            "woT": _bf16(Wo[:, sl].T.reshape(MC, P, D).transpose(1, 0, 2)),
            "bq": np.ascontiguousarray(bq[sl] / SCALE, np.float32),
            "bk": np.ascontiguousarray(bk[sl], np.float32),
            "bv": np.ascontiguousarray(bv[sl], np.float32),
            "valid": validv[b],
        })

    if n_kp not in _BUILD_CACHE:
        _BUILD_CACHE[n_kp] = _build(n_kp)
    nc = _BUILD_CACHE[n_kp]

    res = run_bass_kernel_spmd(nc, in_maps, core_ids=list(range(NCORES)))
    LAST_RESULTS = res

    full = np.zeros((B, S, D), np.float32)
    full += bo.astype(np.float32)
    for c in range(NCORES):
        full[c // GROUPS] += np.asarray(res.results[c]["out"], dtype=np.float32)
    return full
